# revision 17
# baseline (speedup 1.0000x reference)
"""AttentionBlock (GroupNorm(1) + single-head full attention + residual) on 8 TRN2 NeuronCores.

Sharding: data-parallel over batch B=32 -> 4 samples per core; weights replicated.
No collectives needed.

Per-sample dataflow (feature-major "T" = [C_partitions, token_free]):
  xT [C,HW] --bn_stats/PE-colsum--> mean,var -> A,B per-channel -> hT = x*A+B
  qT = qwT.T @ hT          (feature-major)
  kT = kwT.T @ hT          (feature-major)
  v  = hT.T  @ vwT         (token-major  [HW, C])
  wT[y,x]  = kT.T @ qT     (scores, transposed; softmax-max-free: |scores|<~9)
  ew = exp(wT/16)          (ACT, PSUM->SBUF)
  s_bcast[p,x] = ones128.T @ ew   (col-sums replicated on all 128 partitions)
  r_bcast = exp(-ln(s_bcast))     (ACT twice, same table set as Exp)
  oT = v.T @ ew            (feature-major [C, x])
  fT = owT.T @ oT          (feature-major [C_out, x])
  out = x + fT * r_bcast   (DVE)
"""

import numpy as np

import concourse.bass as bass
import concourse.bacc as bacc
import concourse.tile as tile
from concourse import mybir
from concourse.bass_utils import run_bass_kernel_spmd

F32 = mybir.dt.float32
F32R = mybir.dt.float32r
AF = mybir.ActivationFunctionType
OP = mybir.AluOpType

N_CORES = 8
B, C, H, W = 32, 256, 32, 32
HW = H * W          # 1024 tokens
BS = B // N_CORES   # 4 samples per core
CT = C // 128       # 2 channel partition-tiles
NT = HW // 128      # 8 token partition-tiles
EPS = 1e-6
SCALE = C ** -0.5   # 1/16

_PROGRAM_CACHE = {}


def _steer_act_tables(nc):
    """Steer Bacc's act-table-load inserter to the one set that contains every
    activation function this kernel uses (Exp, Ln, Identity, Copy), so a
    single InstLoadActFuncSet is emitted instead of thrashing between the
    per-function first-match sets (~2.7us per reload on ACT)."""
    from concourse.hw_specs import get_activation_tables

    tables = get_activation_tables(nc.m.arch)
    keep = "natural_log_exp_and_others"
    needed = {AF.Exp, AF.Ln, AF.Identity, AF.Copy}
    if keep in tables and needed <= tables[keep]:
        for name, fns in tables.items():
            if name != keep:
                fns -= needed


# Tunables (overridable for A/B experiments via _build_program kwargs)
DEFAULT_CFG = dict(
    ew_bufs=1,        # buffers for the exp(scores) tile (4MB each)
    pp_bufs=2,        # buffers for per-sample pipelined tiles
    ps_mm_bufs=2,     # [128,1024] PSUM slots (2 banks each)
    split_v_psum=True,   # dedicated 1-bank PSUM pool for V projection tiles
    qk_evict_act=True,   # evict q/k projections on ACT (else DVE)
    hw_loop_reps=0,      # bench-only: wrap the body in a hardware For_i loop
    reuse_tags=False,    # alias fin/lns/rbc into dead tiles' pool slots
    ew_bf16=False,       # exp(scores) and v in bf16 (halves ew SBUF, 2x DVE evicts)
    warmup_mms=16,       # dummy matmuls at start to lift the PE HAM clock gate

)


def _build_program(has_vb: bool, has_ob: bool, has_gn: bool = True, reps: int = 1, **cfg_overrides):
    cfg = dict(DEFAULT_CFG, **cfg_overrides)
    nc = bacc.Bacc(
        "TRN2", target_bir_lowering=False, debug=False, enable_asserts=False
    )
    _steer_act_tables(nc)

    x_d = nc.dram_tensor("x", [BS, CT, 128, HW], F32, kind="ExternalInput").ap()
    qwt_d = nc.dram_tensor("qwt", [CT, 128, C], F32R, kind="ExternalInput").ap()
    kwt_d = nc.dram_tensor("kwt", [CT, 128, C], F32R, kind="ExternalInput").ap()
    vwt_d = nc.dram_tensor("vwt", [CT, 128, C], F32R, kind="ExternalInput").ap()
    owt_d = nc.dram_tensor("owt", [CT, 128, C], F32R, kind="ExternalInput").ap()
    gnw_d = nc.dram_tensor("gnw", [CT, 128, 1], F32, kind="ExternalInput").ap()
    gnb_d = nc.dram_tensor("gnb", [CT, 128, 1], F32, kind="ExternalInput").ap()
    qb_d = nc.dram_tensor("qb", [CT, 128, 1], F32, kind="ExternalInput").ap()
    kb_d = nc.dram_tensor("kb", [CT, 128, 1], F32, kind="ExternalInput").ap()
    vb_d = nc.dram_tensor("vb", [1, C], F32, kind="ExternalInput").ap()
    ob_d = nc.dram_tensor("ob", [1, C], F32, kind="ExternalInput").ap()
    out_d = nc.dram_tensor("out", [BS, CT, 128, HW], F32, kind="ExternalOutput").ap()

    with tile.TileContext(nc) as tc:
        with (
            tc.tile_pool(name="consts", bufs=1) as consts,
            tc.tile_pool(name="pp", bufs=cfg["pp_bufs"]) as pp,
            tc.tile_pool(name="ewp", bufs=cfg["ew_bufs"]) as ewp,
            tc.tile_pool(name="small", bufs=2) as small,
            tc.tile_pool(name="ps_mm", bufs=cfg["ps_mm_bufs"], space="PSUM") as ps_mm,
            tc.tile_pool(name="ps_s", bufs=1, space="PSUM") as ps_s,
            tc.tile_pool(name="ps_v", bufs=2, space="PSUM") as ps_v_pool,
        ):
            ps_v = ps_v_pool if cfg["split_v_psum"] else None
            # ---- constants ----
            wq = consts.tile([128, CT, C], F32R)
            wk = consts.tile([128, CT, C], F32R)
            wv = consts.tile([128, CT, C], F32R)
            wo = consts.tile([128, CT, C], F32R)
            for w_sb, w_d in ((wq, qwt_d), (wk, kwt_d), (wv, vwt_d), (wo, owt_d)):
                for kt in range(CT):
                    nc.sync.dma_start(out=w_sb[:, kt, :], in_=w_d[kt])
            gnw = consts.tile([128, CT], F32)
            gnb = consts.tile([128, CT], F32)
            qb_sb = consts.tile([128, CT], F32)
            kb_sb = consts.tile([128, CT], F32)
            for t_sb, t_d in ((gnw, gnw_d), (gnb, gnb_d), (qb_sb, qb_d), (kb_sb, kb_d)):
                for kt in range(CT):
                    nc.sync.dma_start(out=t_sb[:, kt : kt + 1], in_=t_d[kt])
            if has_vb:
                vb_sb = consts.tile([1, C], F32)
                nc.sync.dma_start(out=vb_sb, in_=vb_d)
            if has_ob:
                ob_sb = consts.tile([1, C], F32)
                nc.sync.dma_start(out=ob_sb, in_=ob_d)
            ones = consts.tile([128, 128], F32R)
            ones_f32 = consts.tile([128, 128], F32)
            nc.vector.memset(ones_f32, 1.0)
            nc.vector.tensor_copy(ones, ones_f32)
            ones_row = consts.tile([1, 128], F32)
            nc.vector.memset(ones_row, 1.0)
            ones_col = consts.tile([128, 1], F32)
            nc.vector.memset(ones_col, 1.0)
            eps_sb = consts.tile([1, 1], F32)
            nc.vector.memset(eps_sb, EPS)
            BF16 = mybir.dt.bfloat16
            EWDT = BF16 if cfg["ew_bf16"] else F32R
            if cfg["ew_bf16"]:
                ones_ew = consts.tile([128, 128], BF16)
                nc.vector.memset(ones_ew, 1.0)
            else:
                ones_ew = ones

            hw_loop = cfg.get("hw_loop_reps", 0)
            if cfg["warmup_mms"]:
                warm_ps = ps_s.tile([128, HW], F32, tag="s")
                for i in range(cfg["warmup_mms"]):
                    nc.tensor.matmul(
                        warm_ps[:, 0:128], ones, ones, start=True, stop=True
                    )

            import contextlib
            loop_cm = tc.For_i(0, hw_loop, 1) if hw_loop else contextlib.nullcontext()
            with loop_cm:
             for _rep in range(reps):
              for s in range(BS):
                  # ---- load x ----
                  xt = pp.tile([128, CT, HW], F32, tag="xt")
                  for ct in range(CT):
                      nc.sync.dma_start(out=xt[:, ct, :], in_=x_d[s, ct])

                  # ---- GroupNorm stats (mean/var over all C*HW) ----
                  stats = small.tile([128, CT, 2, 6], F32, tag="stats")
                  mv = small.tile([128, CT, 2], F32, tag="mv")
                  t3 = small.tile([128, CT, 3], F32, tag="t3")
                  for ct in range(CT):
                      for ch in range(2):
                          nc.vector.bn_stats(
                              out=stats[:, ct, ch, :],
                              in_=xt[:, ct, ch * 512 : (ch + 1) * 512],
                          )
                      nc.vector.bn_aggr(out=mv[:, ct, :], in_=stats[:, ct, :, :])
                      # t3 = [mean, var, mean^2] per partition
                      nc.vector.tensor_copy(t3[:, ct, 0:2], mv[:, ct, 0:2])
                      nc.vector.tensor_tensor(
                          t3[:, ct, 2:3], mv[:, ct, 0:1], mv[:, ct, 0:1], OP.mult
                      )
                  stat_ps = (ps_v or ps_mm).tile([1, 3], F32, tag="v" if ps_v else "mm")
                  for ct in range(CT):
                      nc.tensor.matmul(
                          stat_ps,
                          ones_col,
                          t3[:, ct, :],
                          start=(ct == 0),
                          stop=(ct == CT - 1),
                      )
                  st = small.tile([1, 8], F32, tag="st")
                  nc.vector.tensor_copy(st[:, 0:3], stat_ps)
                  # mean = Sm/256 ; var = (Sv + Sm2)/256 - mean^2
                  nc.vector.tensor_scalar(
                      st[:, 3:4], st[:, 0:1], 1.0 / C, 0.0, OP.mult, OP.add
                  )
                  nc.vector.tensor_tensor(st[:, 4:5], st[:, 1:2], st[:, 2:3], OP.add)
                  nc.vector.tensor_scalar(
                      st[:, 5:6], st[:, 4:5], 1.0 / C, 0.0, OP.mult, OP.add
                  )
                  nc.vector.tensor_tensor(st[:, 6:7], st[:, 3:4], st[:, 3:4], OP.mult)
                  nc.vector.tensor_tensor(st[:, 7:8], st[:, 5:6], st[:, 6:7], OP.subtract)
                  mr = small.tile([1, 3], F32, tag="mr")
                  # rstd = exp(-0.5*ln(var+eps)) : Ln and Exp live in one ACT table set
                  nc.scalar.activation(mr[:, 2:3], st[:, 7:8], AF.Ln, bias=eps_sb)
                  nc.scalar.activation(mr[:, 1:2], mr[:, 2:3], AF.Exp, scale=-0.5)
                  nc.vector.tensor_copy(mr[:, 0:1], st[:, 3:4])
                  # broadcast (mean, rstd) to 128 partitions via PE
                  bc_ps = (ps_v or ps_mm).tile([128, 2], F32, tag="v" if ps_v else "mm")
                  nc.tensor.matmul(bc_ps, ones_row, mr[:, 0:2], start=True, stop=True)
                  bc = small.tile([128, 2], F32, tag="bc")
                  nc.vector.tensor_copy(bc, bc_ps)
                  ht = pp.tile([128, CT, HW], F32R, tag="ht")
                  if has_gn:
                      # A = gnw*rstd ; B = gnb - mean*A ; h = x*A + B
                      ab = small.tile([128, CT, 2], F32, tag="ab")
                      for ct in range(CT):
                          nc.vector.tensor_tensor(
                              ab[:, ct, 0:1], gnw[:, ct : ct + 1], bc[:, 1:2], OP.mult
                          )
                          nc.vector.tensor_tensor(
                              ab[:, ct, 1:2], bc[:, 0:1], ab[:, ct, 0:1], OP.mult
                          )
                          nc.vector.tensor_tensor(
                              ab[:, ct, 1:2],
                              gnb[:, ct : ct + 1],
                              ab[:, ct, 1:2],
                              OP.subtract,
                          )
                      for ct in range(CT):
                          nc.vector.tensor_scalar(
                              ht[:, ct, :],
                              xt[:, ct, :],
                              ab[:, ct, 0:1],
                              ab[:, ct, 1:2],
                              OP.mult,
                              OP.add,
                          )
                  else:
                      # gn_w == 1, gn_b == 0: h = (x - mean) * rstd directly
                      for ct in range(CT):
                          nc.vector.tensor_scalar(
                              ht[:, ct, :],
                              xt[:, ct, :],
                              bc[:, 0:1],
                              bc[:, 1:2],
                              OP.subtract,
                              OP.mult,
                          )

                  # ---- Q/K projections (feature-major) ----
                  qT = pp.tile([128, CT, HW], F32R, tag="qT")
                  kT = pp.tile([128, CT, HW], F32R, tag="kT")
                  for dst, w_sb, b_sb in ((qT, wq, qb_sb), (kT, wk, kb_sb)):
                      for ot in range(CT):
                          prj_ps = ps_mm.tile([128, HW], F32, tag="mm")
                          for kt in range(CT):
                              for xb in range(2):
                                  nc.tensor.matmul(
                                      prj_ps[:, xb * 512 : (xb + 1) * 512],
                                      w_sb[:, kt, ot * 128 : (ot + 1) * 128],
                                      ht[:, kt, xb * 512 : (xb + 1) * 512],
                                      start=(kt == 0),
                                      stop=(kt == CT - 1),
                                  )
                          if cfg["qk_evict_act"]:
                              nc.scalar.activation(
                                  dst[:, ot, :],
                                  prj_ps,
                                  AF.Identity,
                                  bias=b_sb[:, ot : ot + 1],
                              )
                          else:
                              nc.vector.tensor_scalar(
                                  dst[:, ot, :],
                                  prj_ps,
                                  b_sb[:, ot : ot + 1],
                                  0.0,
                                  OP.add,
                                  OP.add,
                              )

                  # ---- V projection (token-major) ----
                  v_tok = pp.tile([128, NT, C], EWDT, tag="v_tok")
                  for ng in range(NT // 2):
                      if ps_v is not None:
                          v_ps = ps_v.tile([128, 2, C], F32, tag="v")
                      else:
                          v_ps = ps_mm.tile([128, 2, C], F32, tag="mm")
                      for j in range(2):
                          nt = ng * 2 + j
                          for kt in range(CT):
                              last = kt == CT - 1 and not has_vb
                              nc.tensor.matmul(
                                  v_ps[:, j, :],
                                  ht[:, kt, nt * 128 : (nt + 1) * 128],
                                  wv[:, kt, :],
                                  start=(kt == 0),
                                  stop=last,
                              )
                          if has_vb:
                              nc.tensor.matmul(
                                  v_ps[:, j, :],
                                  ones_row,
                                  vb_sb,
                                  start=False,
                                  stop=True,
                              )
                      nc.vector.tensor_copy(
                          v_tok[:, ng * 2 : ng * 2 + 2, :], v_ps
                      )

                  # ---- scores (transposed) + exp + column sums ----
                  ew = ewp.tile([128, NT, HW], EWDT, tag="ew")
                  s_ps = ps_s.tile([128, HW], F32, tag="s")
                  for yt in range(NT):
                      w_ps = ps_mm.tile([128, HW], F32, tag="mm")
                      for kt in range(CT):
                          for xb in range(2):
                              nc.tensor.matmul(
                                  w_ps[:, xb * 512 : (xb + 1) * 512],
                                  kT[:, kt, yt * 128 : (yt + 1) * 128],
                                  qT[:, kt, xb * 512 : (xb + 1) * 512],
                                  start=(kt == 0),
                                  stop=(kt == CT - 1),
                              )
                      nc.scalar.activation(ew[:, yt, :], w_ps, AF.Exp, scale=SCALE)
                      for xb in range(2):
                          nc.tensor.matmul(
                              s_ps[:, xb * 512 : (xb + 1) * 512],
                              ones_ew,
                              ew[:, yt, xb * 512 : (xb + 1) * 512],
                              start=(yt == 0),
                              stop=(yt == NT - 1),
                          )

                  # r_bcast = exp(-ln(s)) broadcast on all partitions
                  lns = pp.tile([128, HW], F32, tag="qT" if cfg["reuse_tags"] else "lns")
                  rbc = pp.tile([128, HW], F32, tag="kT" if cfg["reuse_tags"] else "rbc")
                  nc.scalar.activation(lns, s_ps, AF.Ln)
                  nc.scalar.activation(rbc, lns, AF.Exp, scale=-1.0)
                  if has_ob:
                      s_row = small.tile([1, HW], F32, tag="s_row")
                      nc.vector.tensor_copy(s_row, s_ps[0:1, :])

                  # ---- attention output (feature-major) ----
                  oT = pp.tile([128, CT, HW], F32R, tag="oT")
                  for ct in range(CT):
                      o_ps = ps_mm.tile([128, HW], F32, tag="mm")
                      for yt in range(NT):
                          for xb in range(2):
                              nc.tensor.matmul(
                                  o_ps[:, xb * 512 : (xb + 1) * 512],
                                  v_tok[:, yt, ct * 128 : (ct + 1) * 128],
                                  ew[:, yt, xb * 512 : (xb + 1) * 512],
                                  start=(yt == 0),
                                  stop=(yt == NT - 1),
                              )
                      nc.vector.tensor_copy(oT[:, ct, :], o_ps)

                  # ---- output projection + normalize + residual ----
                  fin = pp.tile([128, CT, HW], F32, tag="ht" if cfg["reuse_tags"] else "fin")
                  for ot in range(CT):
                      f_ps = ps_mm.tile([128, HW], F32, tag="mm")
                      for kt in range(CT):
                          for xb in range(2):
                              nc.tensor.matmul(
                                  f_ps[:, xb * 512 : (xb + 1) * 512],
                                  wo[:, kt, ot * 128 : (ot + 1) * 128],
                                  oT[:, kt, xb * 512 : (xb + 1) * 512],
                                  start=(kt == 0),
                                  stop=(kt == CT - 1) and not has_ob,
                              )
                      if has_ob:
                          # fT += ob[c] * s[x]; after *r this contributes ob exactly
                          for xb in range(2):
                              nc.tensor.matmul(
                                  f_ps[:, xb * 512 : (xb + 1) * 512],
                                  ob_sb[0:1, ot * 128 : (ot + 1) * 128],
                                  s_row[:, xb * 512 : (xb + 1) * 512],
                                  start=False,
                                  stop=True,
                              )
                      nc.vector.tensor_tensor(fin[:, ot, :], f_ps, rbc, OP.mult)
                      nc.vector.tensor_tensor(
                          fin[:, ot, :], fin[:, ot, :], xt[:, ot, :], OP.add
                      )
                      nc.sync.dma_start(out=out_d[s, ot], in_=fin[:, ot, :])

    nc.compile()
    return nc


def _get_program(has_vb: bool, has_ob: bool, has_gn: bool = True):
    key = (has_vb, has_ob, has_gn)
    if key not in _PROGRAM_CACHE:
        _PROGRAM_CACHE[key] = _build_program(has_vb, has_ob, has_gn=has_gn)
    return _PROGRAM_CACHE[key]


def kernel(x, emb, cond, gn_w, gn_b, qw, qb, kw, kb, vw, vb, ow, ob, **_unused):
    x = np.ascontiguousarray(np.asarray(x, dtype=np.float32))
    f32 = lambda a: np.ascontiguousarray(np.asarray(a, dtype=np.float32))
    gn_w, gn_b = f32(gn_w), f32(gn_b)
    qw, qb, kw, kb = f32(qw), f32(qb), f32(kw), f32(kb)
    vw, vb, ow, ob = f32(vw), f32(vb), f32(ow), f32(ob)

    has_vb = bool(np.any(vb != 0))
    has_ob = bool(np.any(ob != 0))
    has_gn = bool(np.any(gn_w != 1) or np.any(gn_b != 0))
    nc = _get_program(has_vb, has_ob, has_gn)

    wt = lambda w: np.ascontiguousarray(w.T.reshape(CT, 128, C))
    col = lambda b: np.ascontiguousarray(b.reshape(CT, 128, 1))
    row = lambda b: np.ascontiguousarray(b.reshape(1, C))
    shared = {
        "qwt": wt(qw), "kwt": wt(kw), "vwt": wt(vw), "owt": wt(ow),
        "gnw": col(gn_w), "gnb": col(gn_b),
        "qb": col(qb), "kb": col(kb),
        "vb": row(vb), "ob": row(ob),
    }
    in_maps = []
    for i in range(N_CORES):
        m = dict(shared)
        m["x"] = np.ascontiguousarray(
            x[i * BS : (i + 1) * BS].reshape(BS, CT, 128, HW)
        )
        in_maps.append(m)

    res = run_bass_kernel_spmd(nc, in_maps, core_ids=list(range(N_CORES)))
    out = np.concatenate(
        [res.results[i]["out"].reshape(BS, C, H, W) for i in range(N_CORES)], axis=0
    )
    return out

